# revision 9
# baseline (speedup 1.0000x reference)
"""MultiHeadCrossAttention Trainium2 kernel.

Sharding: pure data-parallel — one batch element per NeuronCore (B=8 across
8 cores), weights replicated, no collectives.

Per-core math (N=512 tokens, 12 heads x 128 head-dim):
  qT = Wq^T @ drug^T          (lhsT=Wq tiles, rhs=drug^T)      [INNER, N]
  kT = Wk^T @ target^T                                          [INNER, N]
  v  = target @ Wv            (lhsT=target^T tiles, rhs=Wv)    [N, INNER]
  per head h:
    S  = q_h @ k_h^T + ones^T @ colbias          (colbias = NEG/SCALE on
                                                  masked key columns, folded
                                                  in as a K=1 matmul)
    P  = exp(S*SCALE - rowmax*SCALE)             (ACT, fused rowsum accum)
    Pf = P * (rowkeep/rowsum) + (1-rowkeep)/N    (masked query rows become
                                                  uniform 1/N, matching the
                                                  reference's all-NEG rows)
    O_h^T = V^T @ Pf^T                           (PE transpose of Pf tiles)
  out = O @ Wo + ones^T @ bo + target            (bias as K=1 matmul,
                                                  residual via DVE add)

All matmuls run in bf16 (inputs pre-cast on host), fp32 PSUM accumulation.
Validated numerically: L2 rel err ~5e-4 vs the fp32 reference.
"""

import numpy as np
import ml_dtypes
from contextlib import ExitStack

import concourse.bass as bass
import concourse.mybir as mybir
import concourse.tile as tile
from concourse import bacc
from concourse.masks import make_identity
from concourse.bass_utils import run_bass_kernel_spmd

P = 128
B = 8
N_FULL = 512
DD_FULL = 768
TD_FULL = 2560
H_FULL = 12
D = 128
NEG = -1000000.0


def build_kernel(nc, N=N_FULL, DD=DD_FULL, TD=TD_FULL, H=H_FULL):
    INNER = H * D
    SCALE = D ** (-0.5)
    NT = N // P          # token chunks
    DC = DD // P         # drug-dim chunks
    TC = TD // P         # target-dim chunks
    OSL = min(512, TD)   # out-proj slice
    NO = TD // OSL
    VSL = min(512, INNER)
    NV = INNER // VSL
    f32 = mybir.dt.float32
    bf16 = mybir.dt.bfloat16
    Exp = mybir.ActivationFunctionType.Exp

    drug_bf = nc.dram_tensor("drug_bf", [N, DD], bf16, kind="ExternalInput").ap()
    target_bf = nc.dram_tensor("target_bf", [N, TD], bf16, kind="ExternalInput").ap()
    target_f32 = nc.dram_tensor("target_f32", [N, TD], f32, kind="ExternalInput").ap()
    wq = nc.dram_tensor("wq", [DD, INNER], bf16, kind="ExternalInput").ap()
    wk = nc.dram_tensor("wk", [TD, INNER], bf16, kind="ExternalInput").ap()
    wv = nc.dram_tensor("wv", [TD, INNER], bf16, kind="ExternalInput").ap()
    wo = nc.dram_tensor("wo", [INNER, TD], bf16, kind="ExternalInput").ap()
    bo = nc.dram_tensor("bo", [1, TD], bf16, kind="ExternalInput").ap()
    colbias = nc.dram_tensor("colbias", [1, N], bf16, kind="ExternalInput").ap()
    rowkeep = nc.dram_tensor("rowkeep", [P, NT], f32, kind="ExternalInput").ap()
    rowofs = nc.dram_tensor("rowofs", [P, NT], f32, kind="ExternalInput").ap()
    out = nc.dram_tensor("out", [N, TD], f32, kind="ExternalOutput").ap()

    wq_r = wq.rearrange("(c p) i -> p c i", p=P)
    wk_r = wk.rearrange("(c p) i -> p c i", p=P)
    wv_r = wv.rearrange("(c p) i -> p c i", p=P)
    wo_r = wo.rearrange("(c p) i -> p c i", p=P)

    with tile.TileContext(nc) as tc:
        with ExitStack() as ctx:
            const = ctx.enter_context(tc.tile_pool(name="const", bufs=1))
            res = ctx.enter_context(tc.tile_pool(name="res", bufs=1))
            wpool = ctx.enter_context(tc.tile_pool(name="wpool", bufs=2))
            sm = ctx.enter_context(tc.tile_pool(name="sm", bufs=2))
            st = ctx.enter_context(tc.tile_pool(name="st", bufs=3))
            tg = ctx.enter_context(tc.tile_pool(name="tg", bufs=3))
            psum = ctx.enter_context(tc.tile_pool(name="psum", bufs=2, space="PSUM"))

            identity = const.tile([P, P], bf16, tag="ident")
            make_identity(nc, identity[:])
            ones_t = const.tile([1, P], bf16, tag="ones")
            nc.any.memset(ones_t[:], 1.0)
            colbias_sb = const.tile([1, N], bf16, tag="colbias")
            nc.sync.dma_start(colbias_sb[:], colbias[:])
            bo_sb = const.tile([1, TD], bf16, tag="bo")
            nc.sync.dma_start(bo_sb[:], bo[:])
            rk_sb = const.tile([P, NT], f32, tag="rk")
            nc.sync.dma_start(rk_sb[:], rowkeep[:])
            rofs_sb = const.tile([P, NT], f32, tag="rofs")
            nc.sync.dma_start(rofs_sb[:], rowofs[:])

            # input transposes via PE (bf16): load natural chunks, transpose
            # 128x128 blocks through PSUM, copy back to SBUF
            drugT = [res.tile([P, N], bf16, tag=f"drugT{j}", name=f"drugT{j}")
                     for j in range(DC)]
            targetT = [res.tile([P, N], bf16, tag=f"targetT{j}", name=f"targetT{j}")
                       for j in range(TC)]
            for c in range(NT):
                dnat = wpool.tile([P, DD], bf16, tag="dnat")
                nc.gpsimd.dma_start(dnat[:], drug_bf[c * P:(c + 1) * P, :])
                for j in range(DC):
                    pt = psum.tile([P, P], bf16, tag="pt")
                    nc.tensor.transpose(pt[:], dnat[:, j * P:(j + 1) * P], identity[:])
                    nc.vector.tensor_copy(drugT[j][:, c * P:(c + 1) * P], pt[:])
                tnat = wpool.tile([P, TD], bf16, tag="tnat")
                nc.gpsimd.dma_start(tnat[:], target_bf[c * P:(c + 1) * P, :])
                for j in range(TC):
                    pt = psum.tile([P, P], bf16, tag="pt")
                    nc.tensor.transpose(pt[:], tnat[:, j * P:(j + 1) * P], identity[:])
                    nc.vector.tensor_copy(targetT[j][:, c * P:(c + 1) * P], pt[:])

            # v = target @ Wv  -> [N, INNER], partition = tokens
            v_t = [res.tile([P, INNER], bf16, tag=f"v{t_}", name=f"v{t_}") for t_ in range(NT)]
            for ns in range(NV):
                wv_sb = wpool.tile([P, TC, VSL], bf16, tag="wv")
                nc.gpsimd.dma_start(wv_sb[:], wv_r[:, :, ns * VSL:(ns + 1) * VSL])
                for t_ in range(NT):
                    ps = psum.tile([P, VSL], f32, tag="proj")
                    for kc in range(TC):
                        nc.tensor.matmul(
                            ps[:],
                            lhsT=targetT[kc][:, t_ * P:(t_ + 1) * P],
                            rhs=wv_sb[:, kc, :],
                            start=(kc == 0),
                            stop=(kc == TC - 1),
                        )
                    nc.scalar.copy(v_t[t_][:, ns * VSL:(ns + 1) * VSL], ps[:])

            qT = [res.tile([P, N], bf16, tag=f"qT{h}", name=f"qT{h}") for h in range(H)]
            kT = [res.tile([P, N], bf16, tag=f"kT{h}", name=f"kT{h}") for h in range(H)]
            OT = [res.tile([P, N], bf16, tag=f"OT{h}", name=f"OT{h}") for h in range(H)]

            for h in range(H):
                # qT[h] = (Wq^T drug^T)[h*128:(h+1)*128]
                wq_sb = wpool.tile([P, DC, P], bf16, tag="wq")
                nc.gpsimd.dma_start(wq_sb[:], wq_r[:, :, h * P:(h + 1) * P])
                ps = psum.tile([P, N], f32, tag="proj")
                for kc in range(DC):
                    nc.tensor.matmul(
                        ps[:], lhsT=wq_sb[:, kc, :], rhs=drugT[kc][:],
                        start=(kc == 0), stop=(kc == DC - 1),
                    )
                nc.scalar.copy(qT[h][:], ps[:])

                wk_sb = wpool.tile([P, TC, P], bf16, tag="wk")
                nc.gpsimd.dma_start(wk_sb[:], wk_r[:, :, h * P:(h + 1) * P])
                ps = psum.tile([P, N], f32, tag="proj")
                for kc in range(TC):
                    nc.tensor.matmul(
                        ps[:], lhsT=wk_sb[:, kc, :], rhs=targetT[kc][:],
                        start=(kc == 0), stop=(kc == TC - 1),
                    )
                nc.scalar.copy(kT[h][:], ps[:])

                AT = [sm.tile([P, N], bf16, tag=f"AT{kc}", name=f"AT{kc}") for kc in range(NT)]
                for qc in range(NT):
                    S = psum.tile([P, N], f32, tag="S")
                    nc.tensor.matmul(
                        S[:], lhsT=qT[h][:, qc * P:(qc + 1) * P], rhs=kT[h][:],
                        start=True, stop=False,
                    )
                    nc.tensor.matmul(
                        S[:], lhsT=ones_t[:], rhs=colbias_sb[:],
                        start=False, stop=True,
                    )
                    mx = st.tile([P, 1], f32, tag="mx")
                    nc.vector.reduce_max(mx[:], S[:], axis=mybir.AxisListType.X)
                    nmx = st.tile([P, 1], f32, tag="nmx")
                    nc.vector.tensor_scalar_mul(nmx[:], mx[:], -SCALE)
                    Psb = sm.tile([P, N], bf16, tag="P")
                    rs = st.tile([P, 1], f32, tag="rs")
                    nc.scalar.activation(
                        Psb[:], S[:], Exp, bias=nmx[:], scale=SCALE, accum_out=rs[:]
                    )
                    rcp = st.tile([P, 1], f32, tag="rcp")
                    nc.vector.reciprocal(rcp[:], rs[:])
                    a = st.tile([P, 1], f32, tag="a")
                    nc.vector.tensor_mul(a[:], rcp[:], rk_sb[:, qc:qc + 1])
                    Pf = sm.tile([P, N], bf16, tag="Pf")
                    nc.vector.tensor_scalar(
                        Pf[:], Psb[:], a[:], rofs_sb[:, qc:qc + 1],
                        mybir.AluOpType.mult, mybir.AluOpType.add,
                    )
                    for kc in range(NT):
                        pt = psum.tile([P, P], bf16, tag="pt")
                        nc.tensor.transpose(pt[:], Pf[:, kc * P:(kc + 1) * P], identity[:])
                        nc.scalar.copy(AT[kc][:, qc * P:(qc + 1) * P], pt[:])

                O = psum.tile([P, N], f32, tag="O")
                for kc in range(NT):
                    nc.tensor.matmul(
                        O[:], lhsT=v_t[kc][:, h * D:(h + 1) * D], rhs=AT[kc][:],
                        start=(kc == 0), stop=(kc == NT - 1),
                    )
                nc.scalar.copy(OT[h][:], O[:])

            # out = O @ Wo + bo + target
            for oc in range(NO):
                wo_sb = wpool.tile([P, INNER // P, OSL], bf16, tag="wo")
                nc.gpsimd.dma_start(wo_sb[:], wo_r[:, :, oc * OSL:(oc + 1) * OSL])
                for t_ in range(NT):
                    ps = psum.tile([P, OSL], f32, tag="proj")
                    for ic in range(INNER // P):
                        nc.tensor.matmul(
                            ps[:], lhsT=OT[ic][:, t_ * P:(t_ + 1) * P],
                            rhs=wo_sb[:, ic, :],
                            start=(ic == 0), stop=False,
                        )
                    nc.tensor.matmul(
                        ps[:], lhsT=ones_t[:], rhs=bo_sb[:, oc * OSL:(oc + 1) * OSL],
                        start=False, stop=True,
                    )
                    tgt = tg.tile([P, OSL], f32, tag="tgt")
                    nc.sync.dma_start(
                        tgt[:], target_f32[t_ * P:(t_ + 1) * P, oc * OSL:(oc + 1) * OSL]
                    )
                    ot = tg.tile([P, OSL], f32, tag="ot")
                    nc.vector.tensor_add(ot[:], ps[:], tgt[:])
                    nc.sync.dma_start(
                        out[t_ * P:(t_ + 1) * P, oc * OSL:(oc + 1) * OSL], ot[:]
                    )
    return nc


def make_nc(**kw):
    # Bacc, not plain Bass: nc.compile() runs the wait-legalization passes
    # (generate_event_semaphores et al.) that make the BIR walrus-legal
    nc = bacc.Bacc("TRN2", target_bir_lowering=False, debug=False, num_devices=B)
    build_kernel(nc, **kw)
    nc.compile()
    return nc


def prepare_in_maps(drug, target, drug_mask, pro_mask, Wq, Wk, Wv, Wo, bo,
                    N=N_FULL, H=H_FULL):
    SCALE = D ** (-0.5)
    NT = N // P
    bf = ml_dtypes.bfloat16
    wq_b = np.ascontiguousarray(Wq.astype(bf))
    wk_b = np.ascontiguousarray(Wk.astype(bf))
    wv_b = np.ascontiguousarray(Wv.astype(bf))
    wo_b = np.ascontiguousarray(Wo.astype(bf))
    bo_b = np.ascontiguousarray(bo.reshape(1, -1).astype(bf))
    in_maps = []
    for b in range(drug.shape[0]):
        cb = np.where(pro_mask[b] == 0, NEG / SCALE, 0.0).astype(np.float32)
        rk = (drug_mask[b] != 0).astype(np.float32)
        rofs = (1.0 - rk) / N
        in_maps.append({
            "drug_bf": np.ascontiguousarray(drug[b].astype(bf)),
            "target_bf": np.ascontiguousarray(target[b].astype(bf)),
            "target_f32": np.ascontiguousarray(target[b].astype(np.float32)),
            "wq": wq_b, "wk": wk_b, "wv": wv_b, "wo": wo_b, "bo": bo_b,
            "colbias": np.ascontiguousarray(cb.reshape(1, -1).astype(bf)),
            "rowkeep": np.ascontiguousarray(rk.reshape(NT, P).T.astype(np.float32)),
            "rowofs": np.ascontiguousarray(rofs.reshape(NT, P).T.astype(np.float32)),
        })
    return in_maps


_NC_CACHE = {}
LAST_RESULTS = None


def kernel(drug, target, drug_mask, pro_mask, Wq, Wk, Wv, Wo, bo, **run_kwargs):
    global LAST_RESULTS
    drug = np.asarray(drug, dtype=np.float32)
    target = np.asarray(target, dtype=np.float32)
    drug_mask = np.asarray(drug_mask)
    pro_mask = np.asarray(pro_mask)
    Wq = np.asarray(Wq, dtype=np.float32)
    Wk = np.asarray(Wk, dtype=np.float32)
    Wv = np.asarray(Wv, dtype=np.float32)
    Wo = np.asarray(Wo, dtype=np.float32)
    bo = np.asarray(bo, dtype=np.float32)

    if "nc" not in _NC_CACHE:
        _NC_CACHE["nc"] = make_nc()
    nc = _NC_CACHE["nc"]

    in_maps = prepare_in_maps(drug, target, drug_mask, pro_mask, Wq, Wk, Wv, Wo, bo)
    res = run_bass_kernel_spmd(nc, in_maps, core_ids=list(range(B)), **run_kwargs)
    LAST_RESULTS = res
    return np.stack([res.results[i]["out"] for i in range(B)]).astype(np.float32)
